# revision 24
# baseline (speedup 1.0000x reference)
"""Trainium2 Bass kernel for 4-layer Mamba net (nn_Net_18064632447522).

Sharding: 8 cores = batch(4) x d_inner-half(2). Each core processes one batch
sequence (L=2048) and 512 of the 1024 d_inner channels. Per-layer pair
AllReduces combine x_proj and out_proj partial sums.

Layout on device: activations as [channel(partition), time(free)]. Each layer
runs in two time-halves (LH=1024) to fit SBUF. Selective scan uses the DVE
tensor_tensor_scan instruction per (dblock, state):
  h[:, t] = dA[:, t] * h[:, t-1] + dBx[:, t]
with dA_n = exp(A_n * delta) via ACT Exp(scale=A_n) (n<8) and one DVE multiply
for n>=8 (r^(n+1) = r^n * r). Scan state chains across halves via initial=.
"""

import sys
import numpy as np
import ml_dtypes
BF16NP = ml_dtypes.bfloat16

sys.path.insert(0, "/opt/trn_rl_repo")

import concourse.bass as bass
import concourse.bacc as bacc
from concourse import mybir
from concourse import tile
from concourse.bass_utils import run_bass_kernel_spmd

F32 = mybir.dt.float32
F32R = mybir.dt.float32r
BF16 = mybir.dt.bfloat16
AF = mybir.ActivationFunctionType
OP = mybir.AluOpType

B, L, IN_DIM, OUT_DIM = 4, 2048, 32, 1
HID, NLAYERS = 512, 4
DIN = 2 * HID
DSTATE = 16
DCONV = 4
DTR = 32
EPS = 1e-5
DLOC = DIN // 2          # 512 channels per core
NG = DLOC // 128         # 4 dblocks
HG = HID // 128          # 4 hid blocks
LH = L // 2              # half-sequence
TC = 512                 # matmul time chunk
NTH = LH // TC           # 2 chunks per half

REPLICA_GROUPS = [[0, 1], [2, 3], [4, 5], [6, 7]]


def f32r(ap):
    return ap.bitcast(F32R)


def build_program(a_scales, no_cc=False):
    nc = bacc.Bacc("TRN2", target_bir_lowering=False, debug=False,
                   num_devices=8)
    xT = nc.dram_tensor("xT", [IN_DIM, L], F32, kind="ExternalInput").ap()
    winT = nc.dram_tensor("winT", [IN_DIM, HID], F32, kind="ExternalInput").ap()
    b_in = nc.dram_tensor("b_in", [128, HG], F32, kind="ExternalInput").ap()
    ipw = nc.dram_tensor("ipw", [NLAYERS, 128, HG * 2 * DLOC], BF16,
                         kind="ExternalInput").ap()
    opw = nc.dram_tensor("opw", [NLAYERS, 128, NG * HID], BF16,
                         kind="ExternalInput").ap()
    xpw = nc.dram_tensor("xpw", [NLAYERS, 128, NG * 64], BF16,
                         kind="ExternalInput").ap()
    dtw = nc.dram_tensor("dtw", [NLAYERS, DTR, DLOC], BF16,
                         kind="ExternalInput").ap()
    ppp = nc.dram_tensor("ppp", [NLAYERS, 128, 7 * NG], F32,
                         kind="ExternalInput").ap()
    woutT = nc.dram_tensor("woutT", [128, HG], F32, kind="ExternalInput").ap()
    bout = nc.dram_tensor("bout", [1, 1], F32, kind="ExternalInput").ap()
    identd = nc.dram_tensor("ident", [128, 128], F32, kind="ExternalInput").ap()
    out = nc.dram_tensor("out", [1, L], F32, kind="ExternalOutput").ap()

    with tile.TileContext(nc) as tc:
        _emit(nc, tc, a_scales, xT, winT, b_in, ipw, opw, xpw, dtw, ppp,
              woutT, bout, identd, out, no_cc)
    nc.compile()
    return nc


class _Ctx:
    pass


def _emit(nc, tc, a_scales, xT, winT, b_in, ipw, opw, xpw, dtw, ppp,
          woutT, bout, identd, out, no_cc=False):
    from contextlib import ExitStack
    with ExitStack() as st:
        E = st.enter_context
        persist = E(tc.tile_pool(name="persist", bufs=1))
        wstage = E(tc.tile_pool(name="wstage", bufs=1))
        hpool = E(tc.tile_pool(name="hpool", bufs=1))
        mpool = E(tc.tile_pool(name="mpool", bufs=1))
        actbf = E(tc.tile_pool(name="actbf", bufs=1))
        shortbf = E(tc.tile_pool(name="shortbf", bufs=2))
        dlp = E(tc.tile_pool(name="dl", bufs=4))
        wscp = E(tc.tile_pool(name="wsc", bufs=4))
        dap = E(tc.tile_pool(name="dap", bufs=4))
        q8p = E(tc.tile_pool(name="q8", bufs=4))
        sc3 = E(tc.tile_pool(name="sc3", bufs=2))
        tmpp = E(tc.tile_pool(name="tmpp", bufs=3))
        bcp = E(tc.tile_pool(name="bc", bufs=3))
        zsp = E(tc.tile_pool(name="zs", bufs=2))
        resp = E(tc.tile_pool(name="res", bufs=2))
        smallp = E(tc.tile_pool(name="small", bufs=1))
        hstp = E(tc.tile_pool(name="hstp", bufs=1))
        dramp = E(tc.tile_pool(name="dram", bufs=2, space="DRAM"))

        # ---------- static setup ----------
        id_f = persist.tile([128, 128], F32, tag="identf")
        nc.sync.dma_start(out=id_f[:], in_=identd[:])
        id_bf = persist.tile([128, 128], BF16, tag="ident")
        nc.scalar.activation(id_bf[:], id_f[:], AF.Copy)
        ones_bf = persist.tile([128, 1], BF16, tag="ones")
        nc.scalar.activation(ones_bf[:], id_f[:, 0:1], AF.Copy, bias=1.0,
                             scale=0.0)
        xT_s = persist.tile([IN_DIM, L], F32, tag="xT")
        nc.sync.dma_start(out=xT_s[:], in_=xT[:])
        winT_s = persist.tile([IN_DIM, HID], F32, tag="winT")
        nc.sync.dma_start(out=winT_s[:], in_=winT[:])
        b_in_s = persist.tile([128, HG], F32, tag="b_in")
        nc.sync.dma_start(out=b_in_s[:], in_=b_in[:])
        woutT_s = persist.tile([128, HG], F32, tag="woutT")
        nc.sync.dma_start(out=woutT_s[:], in_=woutT[:])
        bout_s = persist.tile([1, 1], F32, tag="bout")
        nc.sync.dma_start(out=bout_s[:], in_=bout[:])
        eps_s = persist.tile([1, 1], F32, tag="eps")
        nc.gpsimd.memset(eps_s[:], EPS)

        h_t = [hpool.tile([128, L], F32, tag=f"h{j}", name=f"h{j}")
               for j in range(HG)]

        # ---------- input projection ----------
        with tc.tile_pool(name="pmm0", bufs=3, space="PSUM") as pmm:
            for j in range(HG):
                for c in range(L // TC):
                    ps = pmm.tile([128, TC], F32, tag="mm")
                    nc.tensor.matmul(ps[:],
                                     winT_s[:, j * 128:(j + 1) * 128],
                                     xT_s[:, c * TC:(c + 1) * TC],
                                     start=True, stop=True)
                    nc.scalar.activation(h_t[j][:, c * TC:(c + 1) * TC],
                                         ps[:], AF.Identity,
                                         bias=b_in_s[:, j:j + 1])

        # ---------- layers ----------
        def stage_weights(l):
            ipw_s = wstage.tile([128, HG * 2 * DLOC], BF16, tag="ipw",
                                name="ipw_s")
            nc.sync.dma_start(out=ipw_s[:], in_=ipw[l])
            opw_bf = wstage.tile([128, NG * HID], BF16, tag="opwbf",
                                 name="opw_bf")
            nc.sync.dma_start(out=opw_bf[:], in_=opw[l])
            xpw_bf = wstage.tile([128, NG * 64], BF16, tag="xpwbf",
                                 name="xpw_bf")
            nc.sync.dma_start(out=xpw_bf[:], in_=xpw[l])
            dtw_s = wstage.tile([DTR, DLOC], BF16, tag="dtw", name="dtw_s")
            nc.sync.dma_start(out=dtw_s[:], in_=dtw[l])
            ppp_s = wstage.tile([128, 7 * NG], F32, tag="ppp", name="ppp_s")
            nc.sync.dma_start(out=ppp_s[:], in_=ppp[l])
            return ipw_s, opw_bf, xpw_bf, dtw_s, ppp_s

        staged = stage_weights(0)
        for l in range(NLAYERS):
            ipw_s, opw_bf, xpw_bf, dtw_s, ppp_s = staged

            cw_ap = lambda g, i: ppp_s[:, g * 4 + i:g * 4 + i + 1]
            cb_ap = lambda g: ppp_s[:, 4 * NG + g:4 * NG + g + 1]
            db_ap = lambda g: ppp_s[:, 5 * NG + g:5 * NG + g + 1]
            dp_ap = lambda g: ppp_s[:, 6 * NG + g:6 * NG + g + 1]

            hstate = [hstp.tile([128, DSTATE], BF16, tag=f"hst{g}",
                                name=f"hst{g}") for g in range(NG)]
            xs_halo = [hstp.tile([128, DCONV - 1], BF16, tag=f"halo{g}",
                                 name=f"halo{g}") for g in range(NG)]

            for half in range(2):
                t0 = half * LH
                hsl = slice(t0, t0 + LH)

                # ===== phase 1: norm, in_proj, conv, x_proj, dt_proj =====
                with tc.tile_pool(name=f"pA{l}{half}", bufs=4,
                                  space="PSUM") as pmm:
                    # rmsnorm scale row (2*Dsqrt fold in weights)
                    m_b = mpool.tile([128, LH], F32, tag="mb")
                    m_row = smallp.tile([1, LH], F32, tag="mrow")
                    for c in range(NTH):
                        ps = pmm.tile([128, TC], F32, tag="mm")
                        for j in range(HG):
                            sq = shortbf.tile([128, TC], BF16, tag="sq")
                            nc.scalar.activation(
                                sq[:], h_t[j][:, t0 + c * TC:t0 + (c + 1) * TC],
                                AF.Square)
                            nc.tensor.matmul(ps[0:1, :], ones_bf[:], sq[:],
                                             start=(j == 0), stop=(j == HG - 1))
                        lnr = smallp.tile([1, TC], F32, tag="lnr",
                                          name="lnr")
                        nc.scalar.activation(lnr[:], ps[0:1, :], AF.Ln,
                                             scale=1.0 / HID,
                                             bias=eps_s[0:1, 0:1])
                        nc.scalar.activation(m_row[:, c * TC:(c + 1) * TC],
                                             lnr[:], AF.Exp, scale=-0.5)
                    mrow_d = dramp.tile([1, LH], F32, tag="mrowd",
                                        name="mrow_d")
                    nc.sync.dma_start(out=mrow_d[:], in_=m_row[:])
                    nc.sync.dma_start(out=m_b[:],
                                      in_=mrow_d[:].broadcast_to((128, LH)))

                    xs_pre = [wscp.tile([128, LH], BF16, tag="xsp",
                                        name=f"xsp{g}") for g in range(NG)]
                    sz = [actbf.tile([128, LH], BF16, tag=f"sz{g}",
                                     name=f"sz{g}") for g in range(NG)]
                    u_bf, delta = [], []
                    for c in range(NTH):
                        hbf = []
                        for j in range(HG):
                            hb = shortbf.tile([128, TC], BF16, tag="hb",
                                              name="hb", bufs=5)
                            nc.scalar.activation(
                                hb[:], h_t[j][:, t0 + c * TC:
                                              t0 + (c + 1) * TC], AF.Copy)
                            hbf.append(hb)
                        for g in range(NG):
                            ps = pmm.tile([128, TC], F32, tag="mm")
                            for j in range(HG):
                                lt = ipw_s[:, j * 2 * DLOC + g * 128:
                                           j * 2 * DLOC + (g + 1) * 128]
                                nc.tensor.matmul(
                                    ps[:], lt, hbf[j][:],
                                    start=(j == 0), stop=(j == HG - 1))
                            nc.vector.tensor_tensor(
                                xs_pre[g][:, c * TC:(c + 1) * TC], ps[:],
                                m_b[:, c * TC:(c + 1) * TC], OP.mult)
                        for g in range(NG):
                            ps = pmm.tile([128, TC], F32, tag="mm")
                            for j in range(HG):
                                lt = ipw_s[:, j * 2 * DLOC + DLOC + g * 128:
                                           j * 2 * DLOC + DLOC + (g + 1) * 128]
                                nc.tensor.matmul(
                                    ps[:], lt, hbf[j][:],
                                    start=(j == 0), stop=(j == HG - 1))
                            zst = zsp.tile([128, TC], F32, tag="zs")
                            nc.vector.tensor_tensor(
                                zst[:], ps[:], m_b[:, c * TC:(c + 1) * TC],
                                OP.mult)
                            nc.scalar.activation(
                                sz[g][:, c * TC:(c + 1) * TC], zst[:],
                                AF.Silu)

                    # conv + silu
                    for g in range(NG):
                        xc = sc3.tile([128, LH], BF16, tag="xc")
                        nc.vector.tensor_scalar(xc[:], xs_pre[g][:],
                                                cw_ap(g, 3), None, OP.mult)
                        for i in (2, 1, 0):
                            k = 3 - i
                            nc.vector.scalar_tensor_tensor(
                                xc[:, k:LH], xs_pre[g][:, 0:LH - k],
                                cw_ap(g, i), xc[:, k:LH], OP.mult, OP.add)
                            if half == 1:
                                # halo: taps reaching into previous half
                                nc.vector.scalar_tensor_tensor(
                                    xc[:, 0:k], xs_halo[g][:, 3 - k:3],
                                    cw_ap(g, i), xc[:, 0:k], OP.mult, OP.add)
                        ug = actbf.tile([128, LH], BF16, tag=f"u{g}")
                        u_bf.append(ug)
                        nc.scalar.activation(ug[:], xc[:], AF.Silu,
                                             bias=cb_ap(g))
                        if half == 0:
                            nc.vector.tensor_copy(xs_halo[g][:],
                                                  xs_pre[g][:, LH - 3:LH])

                    # x_proj partial + AllReduce
                    cc_in = dramp.tile([64, LH], BF16, tag="ccin")
                    for c in range(NTH):
                        ps = pmm.tile([128, TC], F32, tag="mm")
                        for g in range(NG):
                            nc.tensor.matmul(
                                ps[0:64, :],
                                xpw_bf[:, g * 64:(g + 1) * 64],
                                u_bf[g][:, c * TC:(c + 1) * TC],
                                start=(g == 0), stop=(g == NG - 1))
                        dbp = smallp.tile([64, TC], BF16, tag="dbp",
                                          name="dbp")
                        nc.scalar.activation(dbp[:], ps[0:64, :], AF.Copy)
                        nc.sync.dma_start(out=cc_in[:, c * TC:(c + 1) * TC],
                                          in_=dbp[:])
                    cc_out = dramp.tile([64, LH], BF16, tag="ccout")
                    if no_cc:
                        nc.sync.dma_start(out=cc_out[:], in_=cc_in[:])
                    else:
                        nc.gpsimd.collective_compute(
                            "AllReduce", OP.add,
                            replica_groups=REPLICA_GROUPS,
                            ins=[cc_in.opt()], outs=[cc_out.opt()])
                    dbc_bf = smallp.tile([64, LH], BF16, tag="dbcbf")
                    nc.sync.dma_start(out=dbc_bf[:], in_=cc_out[:])
                    bcd = dramp.tile([32, LH], BF16, tag="bcd", name="bcd")
                    nc.sync.dma_start(out=bcd[:], in_=dbc_bf[32:64, :])

                    # dt_proj + softplus
                    for g in range(NG):
                        dlg = dlp.tile([128, LH], BF16, tag="dl")
                        delta.append(dlg)
                        for c in range(NTH):
                            ps = pmm.tile([128, TC], F32, tag="mm")
                            nc.tensor.matmul(
                                ps[:], dtw_s[:, g * 128:(g + 1) * 128],
                                dbc_bf[0:DTR, c * TC:(c + 1) * TC],
                                start=True, stop=True)
                            es = zsp.tile([128, TC], F32, tag="zs",
                                          name="es")
                            nc.scalar.activation(es[:], ps[:], AF.Exp,
                                                 bias=db_ap(g))
                            nc.scalar.activation(
                                dlg[:, c * TC:(c + 1) * TC], es[:],
                                AF.Ln, bias=1.0)

                # ===== phase 2: scan =====
                w_bf = []
                for g in range(NG):
                    wbf = wscp.tile([128, LH], BF16, tag="wbf")
                    nc.vector.tensor_tensor(wbf[:], delta[g][:], u_bf[g][:],
                                            OP.mult)
                    w_bf.append(wbf)

                with tc.tile_pool(name=f"pY{l}{half}", bufs=1,
                                  space="PSUM") as pyy:
                    y_ps = [pyy.tile([128, LH], F32, tag=f"yps{g}",
                                     name=f"yps{g}") for g in range(NG)]

                    def scan_one(n, g, da):
                        dbx = tmpp.tile([128, LH], BF16, tag="dbx",
                                        name="dbx")
                        nc.vector.tensor_tensor(dbx[:], w_bf[g][:], bbs[n][:],
                                                OP.mult)
                        hsc = tmpp.tile([128, LH], BF16, tag="hsc",
                                        name="hsc")
                        init = (0.0 if half == 0 else hstate[g][:, n:n + 1])
                        nc.vector.tensor_tensor_scan(
                            hsc[:], da[:], dbx[:], init, OP.mult, OP.add)
                        if half == 0:
                            nc.vector.tensor_copy(hstate[g][:, n:n + 1],
                                                  hsc[:, LH - 1:LH])
                        tmp = tmpp.tile([128, LH], BF16, tag="tmp",
                                        name="tmp")
                        eng = nc.gpsimd if (n % 2 == 0) else nc.vector
                        eng.tensor_tensor(tmp[:], hsc[:], cbs[n][:], OP.mult)
                        for c in range(NTH):
                            nc.tensor.matmul(
                                y_ps[g][:, c * TC:(c + 1) * TC],
                                id_bf[:], tmp[:, c * TC:(c + 1) * TC],
                                start=(n == 0 and g is not None and True
                                       and n == 0),
                                stop=(n == DSTATE - 1))

                    bbs, cbs = {}, {}
                    for np_ in range(8):
                        for n in (np_, np_ + 8):
                            bb = bcp.tile([128, LH], BF16, tag="bb",
                                          name="bb")
                            nc.sync.dma_start(
                                out=bb[:],
                                in_=bcd[n:n + 1, :].broadcast_to((128, LH)))
                            bbs[n] = bb
                            cbt = bcp.tile([128, LH], BF16, tag="cbb",
                                           name="cbt")
                            nc.sync.dma_start(
                                out=cbt[:],
                                in_=bcd[DSTATE + n:DSTATE + n + 1, :]
                                .broadcast_to((128, LH)))
                            cbs[n] = cbt
                        for g in range(NG):
                            da = dap.tile([128, LH], BF16, tag="da",
                                          name="da")
                            nc.scalar.activation(
                                da[:], delta[g][:], AF.Exp,
                                scale=float(a_scales[l][np_]))
                            scan_one(np_, g, da)
                            da2 = dap.tile([128, LH], BF16, tag="da",
                                           name="da2")
                            nc.scalar.activation(
                                da2[:], delta[g][:], AF.Exp,
                                scale=float(a_scales[l][np_ + 8]))
                            scan_one(np_ + 8, g, da2)
                    # gate
                    y_sb = []
                    for g in range(NG):
                        yr = resp.tile([128, LH], BF16, tag="yraw")
                        nc.vector.scalar_tensor_tensor(
                            yr[:], u_bf[g][:], dp_ap(g), y_ps[g][:],
                            OP.mult, OP.add)
                        ysb = actbf.tile([128, LH], BF16, tag=f"u{g}",
                                         name=f"ysb{g}")
                        nc.vector.tensor_tensor(ysb[:], yr[:], sz[g][:],
                                                OP.mult)
                        y_sb.append(ysb)

                # ===== phase 3: out_proj + AllReduce + residual =====
                with tc.tile_pool(name=f"pO{l}{half}", bufs=2,
                                  space="PSUM") as poo:
                    oc_in = dramp.tile([HID, LH], BF16, tag="ocin")
                    for c in range(NTH):
                        for j in range(HG):
                            ps = poo.tile([128, TC], F32, tag="ops")
                            for g in range(NG):
                                nc.tensor.matmul(
                                    ps[:],
                                    opw_bf[:, g * HID + j * 128:
                                           g * HID + (j + 1) * 128],
                                    y_sb[g][:, c * TC:(c + 1) * TC],
                                    start=(g == 0), stop=(g == NG - 1))
                            osb = resp.tile([128, TC], BF16, tag="osb",
                                            name="osb")
                            nc.scalar.activation(osb[:], ps[:], AF.Copy)
                            nc.sync.dma_start(
                                out=oc_in[j * 128:(j + 1) * 128,
                                          c * TC:(c + 1) * TC],
                                in_=osb[:])
                    oc_out = dramp.tile([HID, LH], BF16, tag="ocout")
                    if no_cc:
                        nc.sync.dma_start(out=oc_out[:], in_=oc_in[:])
                    else:
                        nc.gpsimd.collective_compute(
                            "AllReduce", OP.add,
                            replica_groups=REPLICA_GROUPS,
                            ins=[oc_in.opt()], outs=[oc_out.opt()])
                    for j in range(HG):
                        for c in range(NTH):
                            onew = resp.tile([128, TC], BF16, tag="onew")
                            nc.sync.dma_start(
                                out=onew[:],
                                in_=oc_out[j * 128:(j + 1) * 128,
                                           c * TC:(c + 1) * TC])
                            hslc = h_t[j][:, t0 + c * TC:t0 + (c + 1) * TC]
                            nc.vector.tensor_tensor(hslc, hslc, onew[:],
                                                    OP.add)

            if l + 1 < NLAYERS:
                staged = stage_weights(l + 1)

        # ---------- output head ----------
        with tc.tile_pool(name="phead", bufs=2, space="PSUM") as ph:
            for c in range(L // TC):
                ps = ph.tile([128, TC], F32, tag="mm")
                for j in range(HG):
                    nc.tensor.matmul(ps[0:1, :],
                                     woutT_s[:, j:j + 1],
                                     h_t[j][:, c * TC:(c + 1) * TC],
                                     start=(j == 0), stop=(j == HG - 1))
                orow = smallp.tile([1, TC], F32, tag="orow", bufs=2)
                nc.scalar.activation(orow[:], ps[0:1, :], AF.Tanh,
                                     bias=bout_s[0:1, 0:1])
                nc.sync.dma_start(out=out[:, c * TC:(c + 1) * TC],
                                  in_=orow[:])


# ======================= host side =======================

def _prep_shards(inputs):
    x = np.asarray(inputs["x"], np.float32)
    w_in = np.asarray(inputs["w_in"], np.float32)
    b_in = np.asarray(inputs["b_in"], np.float32)
    norm_w = np.asarray(inputs["norm_w"], np.float32)
    in_proj_w = np.asarray(inputs["in_proj_w"], np.float32)
    conv_w = np.asarray(inputs["conv_w"], np.float32)
    conv_b = np.asarray(inputs["conv_b"], np.float32)
    xproj_w = np.asarray(inputs["xproj_w"], np.float32)
    dtproj_w = np.asarray(inputs["dtproj_w"], np.float32)
    dtproj_b = np.asarray(inputs["dtproj_b"], np.float32)
    A_log = np.asarray(inputs["A_log"], np.float32)
    D_param = np.asarray(inputs["D_param"], np.float32)
    outproj_w = np.asarray(inputs["outproj_w"], np.float32)
    w_out = np.asarray(inputs["w_out"], np.float32)
    b_out = np.asarray(inputs["b_out"], np.float32)

    A = -np.exp(A_log)
    assert np.ptp(A, axis=1).max() < 1e-5 * np.abs(A).max()
    a_scales = [[float(A[l, 0, n]) for n in range(DSTATE)]
                for l in range(NLAYERS)]

    winT = np.ascontiguousarray(w_in.T)
    b_in_t = np.ascontiguousarray(b_in.reshape(HG, 128).T)
    woutT = np.ascontiguousarray(
        w_out.reshape(OUT_DIM, HG, 128).transpose(2, 1, 0).reshape(128, HG))
    bout_t = b_out.reshape(1, 1)
    ident = np.eye(128, dtype=np.float32)

    def tile_k(mat):
        K, J = mat.shape
        return np.ascontiguousarray(
            mat.reshape(K // 128, 128, J).transpose(1, 0, 2).reshape(128, -1))

    in_maps = []
    for core in range(8):
        b = core // 2
        h2 = core % 2
        dsl = slice(h2 * DLOC, (h2 + 1) * DLOC)
        ipw_l, opw_l, xpw_l, dtw_l, ppp_l = [], [], [], [], []
        for l in range(NLAYERS):
            sel = np.concatenate([in_proj_w[l, dsl, :],
                                  in_proj_w[l, DIN + h2 * DLOC:
                                            DIN + (h2 + 1) * DLOC, :]], 0)
            lt = (sel * norm_w[l][None, :]).T
            ipw_l.append(tile_k(lt).astype(BF16NP))
            opw_l.append(tile_k(np.ascontiguousarray(
                outproj_w[l][:, dsl].T)).astype(BF16NP))
            xpw_l.append(tile_k(np.ascontiguousarray(
                xproj_w[l][:, dsl].T)).astype(BF16NP))
            dtw_l.append(np.ascontiguousarray(
                dtproj_w[l][dsl, :].T).astype(BF16NP))
            pp = np.zeros((128, 7 * NG), np.float32)
            for g in range(NG):
                cs = slice(h2 * DLOC + g * 128, h2 * DLOC + (g + 1) * 128)
                pp[:, g * 4:(g + 1) * 4] = conv_w[l][cs, :]
                pp[:, 4 * NG + g] = conv_b[l][cs]
                pp[:, 5 * NG + g] = dtproj_b[l][cs]
                pp[:, 6 * NG + g] = D_param[l][cs]
            ppp_l.append(pp)
        in_maps.append({
            "xT": np.ascontiguousarray(x[b].T),
            "winT": winT, "b_in": b_in_t,
            "ipw": np.stack(ipw_l), "opw": np.stack(opw_l),
            "xpw": np.stack(xpw_l), "dtw": np.stack(dtw_l),
            "ppp": np.stack(ppp_l),
            "woutT": woutT, "bout": bout_t,
            "ident": ident,
        })
    return in_maps, a_scales


_CACHE = {}


def _build_sharded(nc):
    import jax
    from jax.sharding import Mesh, PartitionSpec
    from jax.experimental.shard_map import shard_map
    import concourse.bass2jax as b2j
    from concourse import mybir as mb

    b2j.install_neuronx_cc_hook()
    partition_name = (nc.partition_id_tensor.name
                      if nc.partition_id_tensor else None)
    in_names, out_names, out_avals = [], [], []
    for alloc in nc.m.functions[0].allocations:
        if not isinstance(alloc, mb.MemoryLocationSet):
            continue
        name = alloc.memorylocations[0].name
        if alloc.kind == "ExternalInput":
            if name != partition_name:
                in_names.append(name)
        elif alloc.kind == "ExternalOutput":
            out_names.append(name)
            out_avals.append(jax.core.ShapedArray(
                tuple(alloc.tensor_shape), mb.dt.np(alloc.dtype)))
    n_params = len(in_names)
    n_outs = len(out_avals)
    in_names_all = list(in_names) + list(out_names)
    if partition_name is not None:
        in_names_all.append(partition_name)

    def _body(*args):
        operands = list(args)
        if partition_name is not None:
            operands.append(b2j.partition_id_tensor())
        return tuple(b2j._bass_exec_p.bind(
            *operands,
            out_avals=tuple(out_avals),
            in_names=tuple(in_names_all),
            out_names=tuple(out_names),
            lowering_input_output_aliases=(),
            sim_require_finite=True,
            sim_require_nnan=True,
            nc=nc,
        ))

    devices = jax.devices()[:8]
    mesh = Mesh(np.asarray(devices), ("core",))
    sharded = jax.jit(
        shard_map(_body, mesh=mesh,
                  in_specs=(PartitionSpec("core"),) * (n_params + n_outs),
                  out_specs=(PartitionSpec("core"),) * n_outs,
                  check_rep=False),
        donate_argnums=tuple(range(n_params, n_params + n_outs)),
        keep_unused=True,
    )
    return sharded, in_names, out_names, out_avals


def _run(in_maps):
    sharded, in_names, out_names, out_avals = _CACHE["exec"]
    concat_in = [np.concatenate([np.asarray(m[name]) for m in in_maps], 0)
                 for name in in_names]
    concat_zeros = [np.zeros((8 * a.shape[0], *a.shape[1:]), a.dtype)
                    for a in out_avals]
    out_arrs = sharded(*concat_in, *concat_zeros)
    return {name: np.asarray(out_arrs[i]).reshape(8, *out_avals[i].shape)
            for i, name in enumerate(out_names)}


def kernel(**inputs):
    in_maps, a_scales = _prep_shards(inputs)
    if "nc" not in _CACHE:
        _CACHE["nc"] = build_program(a_scales)
        _CACHE["exec"] = _build_sharded(_CACHE["nc"])
    res = _run(in_maps)
    outs = []
    for b in range(B):
        o = np.asarray(res["out"][2 * b], np.float32).reshape(L)
        outs.append(o)
    return np.stack(outs, 0).reshape(B, L, 1).ravel().astype(np.float32)


if __name__ == "__main__":
    import reference
    ins = {k: np.asarray(v) for k, v in reference.setup_inputs().items()}
    got = kernel(**ins)
    exp = np.asarray(reference.reference(**ins))
    err = np.abs(got - exp)
    rel = err.max() / (np.abs(exp).max() + 1e-30)
    print("max abs err:", err.max(), "rel:", rel)

